# revision 20
# baseline (speedup 1.0000x reference)
"""Trainium2 Bass kernel for DistangledMultiHeadAttention.

Data-parallel over batch B=8 across 8 NeuronCores (one batch element per
core).  All matmul datapaths run in bf16; PSUM accumulation fp32.

Host-side layout prep: q/k/v/adj are transposed and cast to bf16 on the
host (qT/kT/vT [HID, N], adjT [N, N]), so the device never runs PE
transposes or their PSUM evacuations.

Per batch (use_adj=1), derived from the reference:
    qhT = 0.125*center_N(Wq^T qT)          [HD, N]  (bq cancels in centering)
    khT = center_N(Wk^T kT)                [HD, N]
    vh  = v @ Wv + bv                      [N, HD]  natural (AV stationary)
    uT  = softmax_N(Wu^T kT + bu)          [H, N]
    X_h = (khT_h)^T @ qhT_h                [j, i] == x^T   (per head)
    EM  = exp(X) * adjT                    (ACT exp + DVE mask, 512-wide)
    AV  = [vh_h | 1s]^T @ EM -> psum [128, i]  (rows 64: = s = sum_j EM)
    t2  = (u ⊙ vh)_{pair}^T @ adjT         (PE, pipelined as filler)
    rec = 1/s  (copy rows to SBUF base 0, then custom-DVE recip in place)
    ATT^T = AV[:64]*rec + t2               -> attT SBUF bf16
    out = ATT @ Wo + bo

The main loop is software-pipelined by emission order: the PE stream for
head pair `mo` has the projection and t2 matmuls of `mo+1` woven between
each jo-block's score and AV matmuls, so the PE never waits on the ACT
exp / DVE mask latency and its p-state stays at full clock.  Head drains
(s copy, recip, normalize, +t2) are spread into the following head's
stream.  The output projection reuses the shared PSUM pool so its
matmuls flow right behind the last drain.
"""

import contextlib
import numpy as np
import sys

for _p in ("/opt/trn_rl_repo",):
    if _p not in sys.path:
        sys.path.insert(0, _p)

import ml_dtypes
import concourse.bass as bass
import concourse.mybir as mybir
import concourse.tile as tile
from concourse import bacc
from concourse.masks import make_identity

FP32 = mybir.dt.float32
BF16 = mybir.dt.bfloat16
AF = mybir.ActivationFunctionType
ALU = mybir.AluOpType
P = 128
N, HID, H, D = 1024, 1024, 16, 64
HD = H * D
KO = HID // P
NO = N // P
MO = HD // P
FREE = 512
NIO = N // FREE
HPP = P // D
GS = 4

# Offload wt staging / att+=t2 / ones memset to the (otherwise idle) GPSIMD
# engine.  Standard Pool tensor ops only — no ucode library alternation.
GPSIMD_OFFLOAD = True


def build_core_kernel(use_adj=True):
    """Build the single-core Bass program (SPMD: same program on 8 cores)."""
    nc = bacc.Bacc("TRN2", target_bir_lowering=False, debug=False)

    qT_d = nc.dram_tensor("qT", [HID, N], BF16, kind="ExternalInput")
    kT_d = nc.dram_tensor("kT", [HID, N], BF16, kind="ExternalInput")
    vT_d = nc.dram_tensor("vT", [HID, N], BF16, kind="ExternalInput")
    adjT_d = nc.dram_tensor("adjT", [N, N], BF16, kind="ExternalInput")
    Wq_d = nc.dram_tensor("Wq", [HID, HD], BF16, kind="ExternalInput")
    Wk_d = nc.dram_tensor("Wk", [HID, HD], BF16, kind="ExternalInput")
    Wv_d = nc.dram_tensor("Wv", [HID, HD], BF16, kind="ExternalInput")
    Wu_d = nc.dram_tensor("Wu", [P, KO, H], BF16, kind="ExternalInput")
    Wo_d = nc.dram_tensor("Wo", [HD, HID], BF16, kind="ExternalInput")
    bv_d = nc.dram_tensor("bv", [HD], BF16, kind="ExternalInput")
    bu_d = nc.dram_tensor("bu", [H], FP32, kind="ExternalInput")
    bo_d = nc.dram_tensor("bo", [HID], BF16, kind="ExternalInput")
    out_d = nc.dram_tensor("out", [N, HID], FP32, kind="ExternalOutput")

    scale = float(D) ** (-0.5)
    gp = nc.gpsimd if GPSIMD_OFFLOAD else nc.vector

    with tile.TileContext(nc) as tc:
        with (
            tc.tile_pool(name="persist", bufs=1) as pp,
            tc.tile_pool(name="small", bufs=1) as sp,
            tc.tile_pool(name="meanp", bufs=4) as meanp,
        ):
            qhT = pp.tile([P, MO, N], BF16, tag="qhT")
            khT = pp.tile([P, MO, N], BF16, tag="khT")
            # AV stationary per (jo, h): [vh_h | ones*64] — psum rows 64:128
            # all receive s = sum_j EM (no partition broadcast needed).
            vha = pp.tile([P, NO, H, 2 * D], BF16, tag="vha")
            attT = pp.tile([P, MO, N], BF16, tag="attT")
            WoSB = pp.tile([P, MO, HID], BF16, tag="WoSB")
            kT = pp.tile([P, KO, N], BF16, tag="kT")
            qT = pp.tile([P, KO, N], BF16, tag="qT")
            if use_adj:
                adjT = pp.tile([P, NO, N], BF16, tag="adjT")
                t2sb = pp.tile([P, MO, N], BF16, tag="t2sb")

            bv_bc = sp.tile([P, HD], BF16, tag="bv")
            bo_bc = sp.tile([P, HID], BF16, tag="bo")
            bu_sb = sp.tile([H, 1], FP32, tag="bu")
            unaryT = sp.tile([H, N], BF16, tag="unaryT")
            uT = sp.tile([H, N], BF16, tag="uT")
            u_nat = sp.tile([P, NO, H], BF16, tag="u_nat")
            ident = sp.tile([P, P], BF16, tag="ident")

            for ko in range(KO):
                nc.sync.dma_start(kT[:, ko, :], kT_d[ko * P:(ko + 1) * P, :])
            nc.sync.dma_start(bu_sb[:], bu_d[:, None])
            nc.sync.dma_start(bv_bc[:], bv_d[None, :].to_broadcast((P, HD)))
            nc.sync.dma_start(bo_bc[:], bo_d[None, :].to_broadcast((P, HID)))
            make_identity(nc, ident[:])

            with (
                tc.tile_pool(name="bigp", bufs=3, space="PSUM") as bigp,
                tc.tile_pool(name="xps", bufs=2, space="PSUM") as xps,
            ):
                # ---------- emission helpers (thunk lists) -----------------
                def proj_chunks(xT, W_d, mo, dst, do_scale):
                    """Thunks: 8 x (2 matmuls); last also emits the evac."""
                    st = {}

                    def start():
                        W_sb = wkq.tile([P, KO, P], BF16, tag="wkq",
                                        name="W_sb")
                        nc.sync.dma_start(
                            W_sb[:],
                            W_d[:, mo * P:(mo + 1) * P]
                            .rearrange("(ko p) f -> p ko f", p=P))
                        st["W"] = W_sb
                        st["ps"] = bigp.tile([P, N], FP32, tag="bp",
                                             name="ps")

                    def mm(ko):
                        if ko == 0:
                            start()
                        for io in range(NIO):
                            nc.tensor.matmul(
                                st["ps"][:, io * FREE:(io + 1) * FREE],
                                st["W"][:, ko, :],
                                xT[:, ko, io * FREE:(io + 1) * FREE],
                                start=(ko == 0), stop=(ko == KO - 1))
                        if ko == KO - 1:
                            evac()

                    def evac():
                        ps = st["ps"]
                        sums = meanp.tile([P, 1], FP32, tag="sums",
                                          name="sums")
                        nbias = meanp.tile([P, 1], FP32, tag="nbias",
                                           name="nbias")
                        nc.vector.tensor_reduce(sums[:], ps[:],
                                                mybir.AxisListType.X,
                                                ALU.add)
                        nc.vector.tensor_scalar(nbias[:], sums[:], 1.0 / N,
                                                None, op0=ALU.mult)
                        if do_scale:
                            nc.vector.tensor_scalar(
                                dst[:, mo, :], ps[:], nbias[:], scale,
                                op0=ALU.subtract, op1=ALU.mult)
                        else:
                            nc.vector.tensor_scalar(
                                dst[:, mo, :], ps[:], nbias[:], None,
                                op0=ALU.subtract)

                    return [(lambda ko=ko: mm(ko)) for ko in range(KO)]

                def t2_chunks(mo):
                    """Thunks: 8 x (wt stage + 2 matmuls); last emits evac."""
                    st = {}

                    def mm(jo):
                        if jo == 0:
                            st["ps"] = bigp.tile([P, N], FP32, tag="bp",
                                                 name="pb")
                        wt = wtp.tile([P, HPP, D], BF16, tag="wt", name="wt")
                        gp.tensor_tensor(
                            wt[:],
                            vha[:, jo, mo * HPP:(mo + 1) * HPP, 0:D],
                            u_nat[:, jo, mo * HPP:(mo + 1) * HPP,
                                  None].to_broadcast((P, HPP, D)),
                            ALU.mult)
                        for io in range(NIO):
                            nc.tensor.matmul(
                                st["ps"][:, io * FREE:(io + 1) * FREE],
                                wt[:].rearrange("p h d -> p (h d)"),
                                adjT[:, jo, io * FREE:(io + 1) * FREE],
                                start=(jo == 0), stop=(jo == NO - 1))
                        if jo == NO - 1:
                            for io in range(NIO):
                                nc.scalar.activation(
                                    t2sb[:, mo, io * FREE:(io + 1) * FREE],
                                    st["ps"][:, io * FREE:(io + 1) * FREE],
                                    AF.Copy)

                    return [(lambda jo=jo: mm(jo)) for jo in range(NO)]

                # ---------- per-head emission pieces -----------------------
                def head_scores(h, jo, ems):
                    """Scores jo-block: 2 matmuls + 2 exps + 2 half-masks."""
                    mo, hp = h // HPP, h % HPP
                    em = emp.tile([P, N], BF16, tag="em", name="em")
                    for io in range(NIO):
                        xp = xps.tile([P, FREE], FP32, tag="xp", name="xp")
                        nc.tensor.matmul(
                            xp[:],
                            khT[hp * D:(hp + 1) * D, mo, jo * P:(jo + 1) * P],
                            qhT[hp * D:(hp + 1) * D, mo,
                                io * FREE:(io + 1) * FREE],
                            start=True, stop=True)
                        nc.scalar.activation(
                            em[:, io * FREE:(io + 1) * FREE], xp[:], AF.Exp)
                        if use_adj:
                            nc.vector.tensor_tensor(
                                em[:, io * FREE:(io + 1) * FREE],
                                em[:, io * FREE:(io + 1) * FREE],
                                adjT[:, jo, io * FREE:(io + 1) * FREE],
                                ALU.mult)
                    ems[jo] = em

                def head_av(h, jo, ems, st):
                    if jo == 0:
                        st["pa"] = bigp.tile([P, N], FP32, tag="bp",
                                             name="pa")
                    for io in range(NIO):
                        nc.tensor.matmul(
                            st["pa"][:, io * FREE:(io + 1) * FREE],
                            vha[:, jo, h, :],
                            ems[jo][:, io * FREE:(io + 1) * FREE],
                            start=(jo == 0), stop=(jo == NO - 1))

                def head_drain_thunks(h, st):
                    """Drain: s copy (ACT) -> recip (DVE) -> *rec -> +t2."""
                    mo, hp = h // HPP, h % HPP
                    dst = {}

                    def c0():
                        rec = recp.tile([D, N], FP32, tag="rec", name="rec")
                        nc.vector.tensor_copy(rec[:], st["pa"][D:2 * D, :])
                        dst["rec"] = rec

                    def c1():
                        nc.vector.reciprocal_approx_fast(dst["rec"][:],
                                                         dst["rec"][:])

                    def c2():
                        att = attT[hp * D:(hp + 1) * D, mo, :]
                        nc.vector.tensor_tensor(att, st["pa"][0:D, :],
                                                dst["rec"][:], ALU.mult)

                    def c3():
                        att = attT[hp * D:(hp + 1) * D, mo, :]
                        if use_adj:
                            gp.tensor_tensor(
                                att, att, t2sb[hp * D:(hp + 1) * D, mo, :],
                                ALU.add)
                        else:
                            t2 = sp.tile([D, 1], FP32, tag=f"t2_{h % 4}",
                                         name="t2")
                            pb1 = xps.tile([P, FREE], FP32, tag="xp",
                                           name="pb1")
                            for jo in range(NO):
                                nc.tensor.matmul(
                                    pb1[0:D, 0:1], vha[:, jo, h, 0:D],
                                    u_nat[:, jo, h, None],
                                    start=(jo == 0), stop=(jo == NO - 1))
                            nc.vector.tensor_copy(t2[:], pb1[0:D, 0:1])
                            nc.vector.tensor_tensor(
                                att, att, t2[:].to_broadcast((D, N)),
                                ALU.add)

                    return [c0, c1, c2, c3]

                # ---------- prefix ----------------------------------------
                with (
                    tc.tile_pool(name="prefix", bufs=1) as pfx,
                    tc.tile_pool(name="wup", bufs=1) as wup,
                ):
                    vT = pfx.tile([P, KO, N], BF16, tag="vT")
                    Wv_sb = pfx.tile([P, KO, HD], BF16, tag="Wv")
                    Wu_sb = wup.tile([P, KO, H], BF16, tag="Wu")
                    nc.sync.dma_start(Wu_sb[:], Wu_d[:])
                    for ko in range(KO):
                        nc.sync.dma_start(vT[:, ko, :],
                                          vT_d[ko * P:(ko + 1) * P, :])
                        nc.sync.dma_start(Wv_sb[:, ko, :],
                                          Wv_d[ko * P:(ko + 1) * P, :])
                    if use_adj:
                        for no in range(NO):
                            nc.sync.dma_start(adjT[:, no, :],
                                              adjT_d[no * P:(no + 1) * P, :])
                    for ko in range(KO):
                        nc.sync.dma_start(qT[:, ko, :],
                                          qT_d[ko * P:(ko + 1) * P, :])
                    nc.sync.dma_start(
                        WoSB[:], Wo_d[:].rearrange("(mo p) f -> p mo f", p=P))

                    # ones columns of vh_aug (rows 64:128 of AV psum = s)
                    gp.memset(vha[:, :, :, D:], 1.0)

                    # unary potential u (PE -> ACT -> DVE)
                    for io in range(NIO):
                        up = xps.tile([P, FREE], FP32, tag="xp", name="up")
                        for ko in range(KO):
                            nc.tensor.matmul(
                                up[0:H, :], Wu_sb[:, ko, :],
                                kT[:, ko, io * FREE:(io + 1) * FREE],
                                start=(ko == 0), stop=(ko == KO - 1))
                        nc.scalar.activation(
                            unaryT[:, io * FREE:(io + 1) * FREE],
                            up[0:H, :], AF.Identity, bias=bu_sb[:])
                    usum = sp.tile([H, 1], FP32, tag="usum")
                    urec = sp.tile([H, 1], FP32, tag="urec")
                    nc.scalar.activation(uT[:], unaryT[:], AF.Exp,
                                         accum_out=usum[:])
                    nc.vector.reciprocal(urec[:], usum[:])
                    nc.vector.tensor_scalar(uT[:], uT[:], urec[:], None,
                                            op0=ALU.mult)

                    # v projection: vha = v @ Wv + bv (natural layout);
                    # evac on ACT? tensor_tensor is DVE/Pool — use DVE here
                    # (prefix DVE is light).
                    for nb in range(NO):
                        ps = bigp.tile([P, N], FP32, tag="bp", name="vps")
                        for ko in range(KO):
                            for mf in range(NIO):
                                nc.tensor.matmul(
                                    ps[:, mf * FREE:(mf + 1) * FREE],
                                    vT[:, ko, nb * P:(nb + 1) * P],
                                    Wv_sb[:, ko, mf * FREE:(mf + 1) * FREE],
                                    start=(ko == 0), stop=(ko == KO - 1))
                        for mf in range(NIO):
                            hh = mf * (H // NIO)
                            nc.vector.tensor_tensor(
                                vha[:, nb, hh:hh + H // NIO, 0:D],
                                ps[:, mf * FREE:(mf + 1) * FREE]
                                .rearrange("p (h d) -> p h d", d=D),
                                bv_bc[:, mf * FREE:(mf + 1) * FREE]
                                .rearrange("p (h d) -> p h d", d=D),
                                ALU.add)

                    # u_nat [128, NO, H] via PE transposes of uT
                    for g in range(NO // GS):
                        tp = xps.tile([P, FREE], BF16, tag="xp", name="tpn")
                        for t in range(GS):
                            no = g * GS + t
                            nc.tensor.transpose(
                                tp[:, t * H:(t + 1) * H],
                                uT[:, no * P:(no + 1) * P], ident[:H, :H])
                        nc.scalar.activation(
                            u_nat[:, g * GS:(g + 1) * GS, :],
                            tp[:, :GS * H].rearrange("p (g h) -> p g h",
                                                     g=GS),
                            AF.Copy)

                # ---------- software-pipelined main loop -------------------
                _es = contextlib.ExitStack()
                wkq = _es.enter_context(tc.tile_pool(name="wkq", bufs=4))
                emp = _es.enter_context(tc.tile_pool(name="emp", bufs=3))
                recp = _es.enter_context(tc.tile_pool(name="recp", bufs=2))
                wtp = _es.enter_context(tc.tile_pool(name="wtp", bufs=2 * NO))

                # mo=0 proj + t2 emitted plainly first
                for th in proj_chunks(kT, Wk_d, 0, khT, False):
                    th()
                for th in proj_chunks(qT, Wq_d, 0, qhT, True):
                    th()
                if use_adj:
                    for th in t2_chunks(0):
                        th()

                pending = []          # drain thunks from the previous head

                def pop_pending(k):
                    while k > 0 and pending:
                        pending.pop(0)()
                        k -= 1

                for mo in range(MO):
                    fill = []
                    if mo + 1 < MO:
                        fill += proj_chunks(kT, Wk_d, mo + 1, khT, False)
                        fill += proj_chunks(qT, Wq_d, mo + 1, qhT, True)
                        if use_adj:
                            fill += t2_chunks(mo + 1)
                    fidx = [0]

                    def F(n):
                        while n > 0 and fidx[0] < len(fill):
                            fill[fidx[0]]()
                            fidx[0] += 1
                            n -= 1

                    for hp in range(HPP):
                        h = HPP * mo + hp
                        ems, st = {}, {}
                        head_scores(h, 0, ems)
                        pop_pending(2)
                        for jo in range(NO):
                            if jo + 1 < NO:
                                head_scores(h, jo + 1, ems)
                            F(2 if jo % 2 == 0 else 1)
                            pop_pending(1)
                            head_av(h, jo, ems, st)
                        pending.extend(head_drain_thunks(h, st))
                    F(len(fill))      # flush leftover fillers

                pop_pending(len(pending))

                # ---------- output projection ------------------------------
                with tc.tile_pool(name="outp", bufs=3) as outp:
                    for ic in range(NO):
                        op = bigp.tile([P, N], FP32, tag="bp", name="op")
                        for mo in range(MO):
                            for mf in range(NIO):
                                nc.tensor.matmul(
                                    op[:, mf * FREE:(mf + 1) * FREE],
                                    attT[:, mo, ic * P:(ic + 1) * P],
                                    WoSB[:, mo, mf * FREE:(mf + 1) * FREE],
                                    start=(mo == 0), stop=(mo == MO - 1))
                        outt = outp.tile([P, HID], FP32, tag="outt",
                                         name="outt")
                        for mf in range(NIO):
                            nc.vector.tensor_tensor(
                                outt[:, mf * FREE:(mf + 1) * FREE],
                                op[:, mf * FREE:(mf + 1) * FREE],
                                bo_bc[:, mf * FREE:(mf + 1) * FREE],
                                ALU.add)
                        nc.sync.dma_start(out_d[ic * P:(ic + 1) * P, :],
                                          outt[:])
                _es.close()

    nc.compile()
    return nc


_CACHE = {}


def _get_nc(use_adj: bool):
    key = bool(use_adj)
    if key not in _CACHE:
        _CACHE[key] = build_core_kernel(use_adj=key)
    return _CACHE[key]


def _make_in_maps(ins=None, **kw):
    if ins is None:
        ins = kw
    BF = ml_dtypes.bfloat16
    gf = lambda n: np.ascontiguousarray(np.asarray(ins[n], np.float32))
    gb = lambda n: gf(n).astype(BF)
    q = np.asarray(ins["q"], np.float32)
    k = np.asarray(ins["k"], np.float32)
    v = np.asarray(ins["v"], np.float32)
    adj = np.asarray(ins["adj"], np.float32)
    qT = np.ascontiguousarray(q.transpose(0, 2, 1)).astype(BF)
    kT = np.ascontiguousarray(k.transpose(0, 2, 1)).astype(BF)
    vT = np.ascontiguousarray(v.transpose(0, 2, 1)).astype(BF)
    adjT = np.ascontiguousarray(adj.transpose(0, 2, 1)).astype(BF)
    Wu = np.asarray(ins["Wu"], np.float32)
    shared = {
        "Wq": gb("Wq"), "Wk": gb("Wk"), "Wv": gb("Wv"), "Wo": gb("Wo"),
        "Wu": np.ascontiguousarray(
            Wu.reshape(KO, P, H).transpose(1, 0, 2)).astype(BF),
        "bv": gb("bv"), "bu": gf("bu"), "bo": gb("bo"),
    }
    in_maps = []
    for b in range(q.shape[0]):
        m = dict(shared)
        m["qT"], m["kT"], m["vT"], m["adjT"] = qT[b], kT[b], vT[b], adjT[b]
        in_maps.append(m)
    return in_maps


def kernel(q, k, v, adj, use_adj, Wq, bq, Wk, bk, Wv, bv, Wu, bu, Wo, bo):
    from concourse.bass_utils import run_bass_kernel_spmd

    nc = _get_nc(bool(int(np.asarray(use_adj))))
    in_maps = _make_in_maps(q=q, k=k, v=v, adj=adj, Wq=Wq, Wk=Wk, Wv=Wv,
                            Wu=Wu, Wo=Wo, bv=bv, bu=bu, bo=bo)
    res = run_bass_kernel_spmd(nc, in_maps, list(range(len(in_maps))))
    return np.stack([res.results[b]["out"] for b in range(len(in_maps))],
                    axis=0)


# revision 21
# speedup vs baseline: 1.2382x; 1.2382x over previous
"""Trainium2 Bass kernel for DistangledMultiHeadAttention.

Data-parallel over batch B=8 across 8 NeuronCores (one batch element per
core).  All matmul datapaths run in bf16; PSUM accumulation fp32.

Host-side layout prep: q/k/v/adj are transposed and cast to bf16 on the
host (qT/kT/vT [HID, N], adjT [N, N]), so the device never runs PE
transposes or their PSUM evacuations.

Per batch (use_adj=1), derived from the reference:
    qhT = 0.125*center_N(Wq^T qT)          [HD, N]  (bq cancels in centering)
    khT = center_N(Wk^T kT)                [HD, N]
    vh  = v @ Wv + bv                      [N, HD]  natural (AV stationary)
    uT  = softmax_N(Wu^T kT + bu)          [H, N]
    X_h = (khT_h)^T @ qhT_h                [j, i] == x^T   (per head)
    EM  = exp(X) * adjT                    (ACT exp + DVE mask, 512-wide)
    AV  = [vh_h | 1s]^T @ EM -> psum [128, i]  (rows 64: = s = sum_j EM)
    t2  = (u ⊙ vh)_{pair}^T @ adjT         (PE, pipelined as filler)
    rec = 1/s  (copy rows to SBUF base 0, then custom-DVE recip in place)
    ATT^T = AV[:64]*rec + t2               -> attT SBUF bf16
    out = ATT @ Wo + bo

The main loop is software-pipelined by emission order: the PE stream for
head pair `mo` has the projection and t2 matmuls of `mo+1` woven between
each jo-block's score and AV matmuls, so the PE never waits on the ACT
exp / DVE mask latency and its p-state stays at full clock.  Head drains
(s copy, recip, normalize, +t2) are spread into the following head's
stream.  The output projection reuses the shared PSUM pool so its
matmuls flow right behind the last drain.
"""

import contextlib
import numpy as np
import sys

for _p in ("/opt/trn_rl_repo",):
    if _p not in sys.path:
        sys.path.insert(0, _p)

import ml_dtypes
import concourse.bass as bass
import concourse.mybir as mybir
import concourse.tile as tile
from concourse import bacc
from concourse.masks import make_identity

FP32 = mybir.dt.float32
BF16 = mybir.dt.bfloat16
AF = mybir.ActivationFunctionType
ALU = mybir.AluOpType
P = 128
N, HID, H, D = 1024, 1024, 16, 64
HD = H * D
KO = HID // P
NO = N // P
MO = HD // P
FREE = 512
NIO = N // FREE
HPP = P // D
GS = 4

# Offload wt staging / att+=t2 / ones memset to the (otherwise idle) GPSIMD
# engine.  Standard Pool tensor ops only — no ucode library alternation.
GPSIMD_OFFLOAD = False


def build_core_kernel(use_adj=True):
    """Build the single-core Bass program (SPMD: same program on 8 cores)."""
    nc = bacc.Bacc("TRN2", target_bir_lowering=False, debug=False)

    qT_d = nc.dram_tensor("qT", [HID, N], BF16, kind="ExternalInput")
    kT_d = nc.dram_tensor("kT", [HID, N], BF16, kind="ExternalInput")
    vT_d = nc.dram_tensor("vT", [HID, N], BF16, kind="ExternalInput")
    adjT_d = nc.dram_tensor("adjT", [N, N], BF16, kind="ExternalInput")
    Wq_d = nc.dram_tensor("Wq", [HID, HD], BF16, kind="ExternalInput")
    Wk_d = nc.dram_tensor("Wk", [HID, HD], BF16, kind="ExternalInput")
    Wv_d = nc.dram_tensor("Wv", [HID, HD], BF16, kind="ExternalInput")
    Wu_d = nc.dram_tensor("Wu", [P, KO, H], BF16, kind="ExternalInput")
    Wo_d = nc.dram_tensor("Wo", [HD, HID], BF16, kind="ExternalInput")
    bv_d = nc.dram_tensor("bv", [HD], BF16, kind="ExternalInput")
    bu_d = nc.dram_tensor("bu", [H], FP32, kind="ExternalInput")
    bo_d = nc.dram_tensor("bo", [HID], BF16, kind="ExternalInput")
    out_d = nc.dram_tensor("out", [N, HID], FP32, kind="ExternalOutput")

    scale = float(D) ** (-0.5)
    gp = nc.gpsimd if GPSIMD_OFFLOAD else nc.vector

    with tile.TileContext(nc) as tc:
        with (
            tc.tile_pool(name="persist", bufs=1) as pp,
            tc.tile_pool(name="small", bufs=1) as sp,
            tc.tile_pool(name="meanp", bufs=4) as meanp,
        ):
            qhT = pp.tile([P, MO, N], BF16, tag="qhT")
            khT = pp.tile([P, MO, N], BF16, tag="khT")
            # AV stationary per (jo, h): [vh_h | ones*64] — psum rows 64:128
            # all receive s = sum_j EM (no partition broadcast needed).
            vha = pp.tile([P, NO, H, 2 * D], BF16, tag="vha")
            attT = pp.tile([P, MO, N], BF16, tag="attT")
            WoSB = pp.tile([P, MO, HID], BF16, tag="WoSB")
            kT = pp.tile([P, KO, N], BF16, tag="kT")
            qT = pp.tile([P, KO, N], BF16, tag="qT")
            if use_adj:
                adjT = pp.tile([P, NO, N], BF16, tag="adjT")
                t2sb = pp.tile([P, MO, N], BF16, tag="t2sb")

            bv_bc = sp.tile([P, HD], BF16, tag="bv")
            bo_bc = sp.tile([P, HID], BF16, tag="bo")
            bu_sb = sp.tile([H, 1], FP32, tag="bu")
            unaryT = sp.tile([H, N], BF16, tag="unaryT")
            uT = sp.tile([H, N], BF16, tag="uT")
            u_nat = sp.tile([P, NO, H], BF16, tag="u_nat")
            ident = sp.tile([P, P], BF16, tag="ident")

            for ko in range(KO):
                nc.sync.dma_start(kT[:, ko, :], kT_d[ko * P:(ko + 1) * P, :])
            nc.sync.dma_start(bu_sb[:], bu_d[:, None])
            nc.sync.dma_start(bv_bc[:], bv_d[None, :].to_broadcast((P, HD)))
            nc.sync.dma_start(bo_bc[:], bo_d[None, :].to_broadcast((P, HID)))
            make_identity(nc, ident[:])

            with (
                tc.tile_pool(name="bigp", bufs=3, space="PSUM") as bigp,
                tc.tile_pool(name="xps", bufs=2, space="PSUM") as xps,
            ):
                # ---------- emission helpers (thunk lists) -----------------
                def proj_chunks(xT, W_d, mo, dst, do_scale):
                    """Thunks: 8 x (2 matmuls); last also emits the evac."""
                    st = {}

                    def start():
                        W_sb = wkq.tile([P, KO, P], BF16, tag="wkq",
                                        name="W_sb")
                        nc.sync.dma_start(
                            W_sb[:],
                            W_d[:, mo * P:(mo + 1) * P]
                            .rearrange("(ko p) f -> p ko f", p=P))
                        st["W"] = W_sb
                        st["ps"] = bigp.tile([P, N], FP32, tag="bp",
                                             name="ps")

                    def mm(ko):
                        if ko == 0:
                            start()
                        for io in range(NIO):
                            nc.tensor.matmul(
                                st["ps"][:, io * FREE:(io + 1) * FREE],
                                st["W"][:, ko, :],
                                xT[:, ko, io * FREE:(io + 1) * FREE],
                                start=(ko == 0), stop=(ko == KO - 1))
                        if ko == KO - 1:
                            evac()

                    def evac():
                        ps = st["ps"]
                        sums = meanp.tile([P, 1], FP32, tag="sums",
                                          name="sums")
                        nbias = meanp.tile([P, 1], FP32, tag="nbias",
                                           name="nbias")
                        nc.vector.tensor_reduce(sums[:], ps[:],
                                                mybir.AxisListType.X,
                                                ALU.add)
                        nc.vector.tensor_scalar(nbias[:], sums[:], 1.0 / N,
                                                None, op0=ALU.mult)
                        if do_scale:
                            nc.vector.tensor_scalar(
                                dst[:, mo, :], ps[:], nbias[:], scale,
                                op0=ALU.subtract, op1=ALU.mult)
                        else:
                            nc.vector.tensor_scalar(
                                dst[:, mo, :], ps[:], nbias[:], None,
                                op0=ALU.subtract)

                    return [(lambda ko=ko: mm(ko)) for ko in range(KO)]

                def t2_chunks(mo):
                    """Thunks: 8 x (wt stage + 2 matmuls); last emits evac."""
                    st = {}

                    def mm(jo):
                        if jo == 0:
                            st["ps"] = bigp.tile([P, N], FP32, tag="bp",
                                                 name="pb")
                        wt = wtp.tile([P, HPP, D], BF16, tag="wt", name="wt")
                        gp.tensor_tensor(
                            wt[:],
                            vha[:, jo, mo * HPP:(mo + 1) * HPP, 0:D],
                            u_nat[:, jo, mo * HPP:(mo + 1) * HPP,
                                  None].to_broadcast((P, HPP, D)),
                            ALU.mult)
                        for io in range(NIO):
                            nc.tensor.matmul(
                                st["ps"][:, io * FREE:(io + 1) * FREE],
                                wt[:].rearrange("p h d -> p (h d)"),
                                adjT[:, jo, io * FREE:(io + 1) * FREE],
                                start=(jo == 0), stop=(jo == NO - 1))
                        if jo == NO - 1:
                            for io in range(NIO):
                                nc.scalar.activation(
                                    t2sb[:, mo, io * FREE:(io + 1) * FREE],
                                    st["ps"][:, io * FREE:(io + 1) * FREE],
                                    AF.Copy)

                    return [(lambda jo=jo: mm(jo)) for jo in range(NO)]

                # ---------- per-head emission pieces -----------------------
                def head_scores(h, jo, ems):
                    """Scores jo-block: 2 matmuls + 2 exps + 2 half-masks."""
                    mo, hp = h // HPP, h % HPP
                    em = emp.tile([P, N], BF16, tag="em", name="em")
                    for io in range(NIO):
                        xp = xps.tile([P, FREE], FP32, tag="xp", name="xp")
                        nc.tensor.matmul(
                            xp[:],
                            khT[hp * D:(hp + 1) * D, mo, jo * P:(jo + 1) * P],
                            qhT[hp * D:(hp + 1) * D, mo,
                                io * FREE:(io + 1) * FREE],
                            start=True, stop=True)
                        nc.scalar.activation(
                            em[:, io * FREE:(io + 1) * FREE], xp[:], AF.Exp)
                        if use_adj:
                            nc.vector.tensor_tensor(
                                em[:, io * FREE:(io + 1) * FREE],
                                em[:, io * FREE:(io + 1) * FREE],
                                adjT[:, jo, io * FREE:(io + 1) * FREE],
                                ALU.mult)
                    ems[jo] = em

                def head_av(h, jo, ems, st):
                    if jo == 0:
                        st["pa"] = bigp.tile([P, N], FP32, tag="bp",
                                             name="pa")
                    for io in range(NIO):
                        nc.tensor.matmul(
                            st["pa"][:, io * FREE:(io + 1) * FREE],
                            vha[:, jo, h, :],
                            ems[jo][:, io * FREE:(io + 1) * FREE],
                            start=(jo == 0), stop=(jo == NO - 1))

                def head_drain_thunks(h, st):
                    """Drain: s copy (ACT) -> recip (DVE) -> *rec -> +t2."""
                    mo, hp = h // HPP, h % HPP
                    dst = {}

                    def c0():
                        rec = recp.tile([D, N], FP32, tag="rec", name="rec")
                        nc.vector.tensor_copy(rec[:], st["pa"][D:2 * D, :])
                        dst["rec"] = rec

                    def c1():
                        nc.vector.reciprocal_approx_fast(dst["rec"][:],
                                                         dst["rec"][:])

                    def c2():
                        att = attT[hp * D:(hp + 1) * D, mo, :]
                        nc.vector.tensor_tensor(att, st["pa"][0:D, :],
                                                dst["rec"][:], ALU.mult)

                    def c3():
                        att = attT[hp * D:(hp + 1) * D, mo, :]
                        if use_adj:
                            gp.tensor_tensor(
                                att, att, t2sb[hp * D:(hp + 1) * D, mo, :],
                                ALU.add)
                        else:
                            t2 = sp.tile([D, 1], FP32, tag=f"t2_{h % 4}",
                                         name="t2")
                            pb1 = xps.tile([P, FREE], FP32, tag="xp",
                                           name="pb1")
                            for jo in range(NO):
                                nc.tensor.matmul(
                                    pb1[0:D, 0:1], vha[:, jo, h, 0:D],
                                    u_nat[:, jo, h, None],
                                    start=(jo == 0), stop=(jo == NO - 1))
                            nc.vector.tensor_copy(t2[:], pb1[0:D, 0:1])
                            nc.vector.tensor_tensor(
                                att, att, t2[:].to_broadcast((D, N)),
                                ALU.add)

                    return [c0, c1, c2, c3]

                # ---------- prefix ----------------------------------------
                with (
                    tc.tile_pool(name="prefix", bufs=1) as pfx,
                    tc.tile_pool(name="wup", bufs=1) as wup,
                ):
                    vT = pfx.tile([P, KO, N], BF16, tag="vT")
                    Wv_sb = pfx.tile([P, KO, HD], BF16, tag="Wv")
                    Wu_sb = wup.tile([P, KO, H], BF16, tag="Wu")
                    nc.sync.dma_start(Wu_sb[:], Wu_d[:])
                    for ko in range(KO):
                        nc.sync.dma_start(vT[:, ko, :],
                                          vT_d[ko * P:(ko + 1) * P, :])
                        nc.sync.dma_start(Wv_sb[:, ko, :],
                                          Wv_d[ko * P:(ko + 1) * P, :])
                    if use_adj:
                        for no in range(NO):
                            nc.sync.dma_start(adjT[:, no, :],
                                              adjT_d[no * P:(no + 1) * P, :])
                    for ko in range(KO):
                        nc.sync.dma_start(qT[:, ko, :],
                                          qT_d[ko * P:(ko + 1) * P, :])
                    nc.sync.dma_start(
                        WoSB[:], Wo_d[:].rearrange("(mo p) f -> p mo f", p=P))

                    # ones columns of vh_aug (rows 64:128 of AV psum = s)
                    gp.memset(vha[:, :, :, D:], 1.0)

                    # unary potential u (PE -> ACT -> DVE)
                    for io in range(NIO):
                        up = xps.tile([P, FREE], FP32, tag="xp", name="up")
                        for ko in range(KO):
                            nc.tensor.matmul(
                                up[0:H, :], Wu_sb[:, ko, :],
                                kT[:, ko, io * FREE:(io + 1) * FREE],
                                start=(ko == 0), stop=(ko == KO - 1))
                        nc.scalar.activation(
                            unaryT[:, io * FREE:(io + 1) * FREE],
                            up[0:H, :], AF.Identity, bias=bu_sb[:])
                    usum = sp.tile([H, 1], FP32, tag="usum")
                    urec = sp.tile([H, 1], FP32, tag="urec")
                    nc.scalar.activation(uT[:], unaryT[:], AF.Exp,
                                         accum_out=usum[:])
                    nc.vector.reciprocal(urec[:], usum[:])
                    nc.vector.tensor_scalar(uT[:], uT[:], urec[:], None,
                                            op0=ALU.mult)

                    # v projection: vha = v @ Wv + bv (natural layout);
                    # evac on ACT? tensor_tensor is DVE/Pool — use DVE here
                    # (prefix DVE is light).
                    for nb in range(NO):
                        ps = bigp.tile([P, N], FP32, tag="bp", name="vps")
                        for ko in range(KO):
                            for mf in range(NIO):
                                nc.tensor.matmul(
                                    ps[:, mf * FREE:(mf + 1) * FREE],
                                    vT[:, ko, nb * P:(nb + 1) * P],
                                    Wv_sb[:, ko, mf * FREE:(mf + 1) * FREE],
                                    start=(ko == 0), stop=(ko == KO - 1))
                        for mf in range(NIO):
                            hh = mf * (H // NIO)
                            nc.vector.tensor_tensor(
                                vha[:, nb, hh:hh + H // NIO, 0:D],
                                ps[:, mf * FREE:(mf + 1) * FREE]
                                .rearrange("p (h d) -> p h d", d=D),
                                bv_bc[:, mf * FREE:(mf + 1) * FREE]
                                .rearrange("p (h d) -> p h d", d=D),
                                ALU.add)

                    # u_nat [128, NO, H] via PE transposes of uT
                    for g in range(NO // GS):
                        tp = xps.tile([P, FREE], BF16, tag="xp", name="tpn")
                        for t in range(GS):
                            no = g * GS + t
                            nc.tensor.transpose(
                                tp[:, t * H:(t + 1) * H],
                                uT[:, no * P:(no + 1) * P], ident[:H, :H])
                        nc.scalar.activation(
                            u_nat[:, g * GS:(g + 1) * GS, :],
                            tp[:, :GS * H].rearrange("p (g h) -> p g h",
                                                     g=GS),
                            AF.Copy)

                # ---------- software-pipelined main loop -------------------
                _es = contextlib.ExitStack()
                wkq = _es.enter_context(tc.tile_pool(name="wkq", bufs=4))
                emp = _es.enter_context(tc.tile_pool(name="emp", bufs=3))
                recp = _es.enter_context(tc.tile_pool(name="recp", bufs=2))
                wtp = _es.enter_context(tc.tile_pool(name="wtp", bufs=2 * NO))

                # mo=0 proj + t2 emitted plainly first
                for th in proj_chunks(kT, Wk_d, 0, khT, False):
                    th()
                for th in proj_chunks(qT, Wq_d, 0, qhT, True):
                    th()
                if use_adj:
                    for th in t2_chunks(0):
                        th()

                pending = []          # drain thunks from the previous head

                def pop_pending(k):
                    while k > 0 and pending:
                        pending.pop(0)()
                        k -= 1

                for mo in range(MO):
                    fill = []
                    if mo + 1 < MO:
                        fill += proj_chunks(kT, Wk_d, mo + 1, khT, False)
                        fill += proj_chunks(qT, Wq_d, mo + 1, qhT, True)
                        if use_adj:
                            fill += t2_chunks(mo + 1)
                    fidx = [0]

                    def F(n):
                        while n > 0 and fidx[0] < len(fill):
                            fill[fidx[0]]()
                            fidx[0] += 1
                            n -= 1

                    for hp in range(HPP):
                        h = HPP * mo + hp
                        ems, st = {}, {}
                        head_scores(h, 0, ems)
                        pop_pending(2)
                        for jo in range(NO):
                            if jo + 1 < NO:
                                head_scores(h, jo + 1, ems)
                            F(2 if jo % 2 == 0 else 1)
                            pop_pending(1)
                            head_av(h, jo, ems, st)
                        pending.extend(head_drain_thunks(h, st))
                    F(len(fill))      # flush leftover fillers

                pop_pending(len(pending))

                # ---------- output projection ------------------------------
                with tc.tile_pool(name="outp", bufs=3) as outp:
                    for ic in range(NO):
                        op = bigp.tile([P, N], FP32, tag="bp", name="op")
                        for mo in range(MO):
                            for mf in range(NIO):
                                nc.tensor.matmul(
                                    op[:, mf * FREE:(mf + 1) * FREE],
                                    attT[:, mo, ic * P:(ic + 1) * P],
                                    WoSB[:, mo, mf * FREE:(mf + 1) * FREE],
                                    start=(mo == 0), stop=(mo == MO - 1))
                        outt = outp.tile([P, HID], FP32, tag="outt",
                                         name="outt")
                        for mf in range(NIO):
                            nc.vector.tensor_tensor(
                                outt[:, mf * FREE:(mf + 1) * FREE],
                                op[:, mf * FREE:(mf + 1) * FREE],
                                bo_bc[:, mf * FREE:(mf + 1) * FREE],
                                ALU.add)
                        nc.sync.dma_start(out_d[ic * P:(ic + 1) * P, :],
                                          outt[:])
                _es.close()

    nc.compile()
    return nc


_CACHE = {}


def _get_nc(use_adj: bool):
    key = bool(use_adj)
    if key not in _CACHE:
        _CACHE[key] = build_core_kernel(use_adj=key)
    return _CACHE[key]


def _make_in_maps(ins=None, **kw):
    if ins is None:
        ins = kw
    BF = ml_dtypes.bfloat16
    gf = lambda n: np.ascontiguousarray(np.asarray(ins[n], np.float32))
    gb = lambda n: gf(n).astype(BF)
    q = np.asarray(ins["q"], np.float32)
    k = np.asarray(ins["k"], np.float32)
    v = np.asarray(ins["v"], np.float32)
    adj = np.asarray(ins["adj"], np.float32)
    qT = np.ascontiguousarray(q.transpose(0, 2, 1)).astype(BF)
    kT = np.ascontiguousarray(k.transpose(0, 2, 1)).astype(BF)
    vT = np.ascontiguousarray(v.transpose(0, 2, 1)).astype(BF)
    adjT = np.ascontiguousarray(adj.transpose(0, 2, 1)).astype(BF)
    Wu = np.asarray(ins["Wu"], np.float32)
    shared = {
        "Wq": gb("Wq"), "Wk": gb("Wk"), "Wv": gb("Wv"), "Wo": gb("Wo"),
        "Wu": np.ascontiguousarray(
            Wu.reshape(KO, P, H).transpose(1, 0, 2)).astype(BF),
        "bv": gb("bv"), "bu": gf("bu"), "bo": gb("bo"),
    }
    in_maps = []
    for b in range(q.shape[0]):
        m = dict(shared)
        m["qT"], m["kT"], m["vT"], m["adjT"] = qT[b], kT[b], vT[b], adjT[b]
        in_maps.append(m)
    return in_maps


def kernel(q, k, v, adj, use_adj, Wq, bq, Wk, bk, Wv, bv, Wu, bu, Wo, bo):
    from concourse.bass_utils import run_bass_kernel_spmd

    nc = _get_nc(bool(int(np.asarray(use_adj))))
    in_maps = _make_in_maps(q=q, k=k, v=v, adj=adj, Wq=Wq, Wk=Wk, Wv=Wv,
                            Wu=Wu, Wo=Wo, bv=bv, bu=bu, bo=bo)
    res = run_bass_kernel_spmd(nc, in_maps, list(range(len(in_maps))))
    return np.stack([res.results[b]["out"] for b in range(len(in_maps))],
                    axis=0)
